# revision 1
# baseline (speedup 1.0000x reference)
"""Trainium2 Bass kernel for nn_BidirectionalAttention (B=2, N=2048, D=2048, H=16).

Head-parallel tensor sharding across 8 NeuronCores (2 heads/core):
  phase A: qkv projection from x^T (rope applied on natural layout, then
           PE-transpose q,k into [head_dim, seq] layout), intermediates to DRAM
  phase B: per (batch, head): transposed attention scores s^T[k,q] = k^T.T @ q^T,
           exp on ScalarE, softmax denominator via ones-matmul partition sum
           (broadcast back to 128 partitions with a K=1 matmul, then fast
           reciprocal), unnormalized attn @ v accumulated transposed, scaled
  phase C: output projection partial = av^T.T @ wo_rows per core (interleaved
           with phase B per batch; av stays SBUF-resident)
Host: shard/transpose/pre-round inputs, sum the 8 partial outputs (the
"all-reduce after wo" done at gather time).

Matmuls run in float32r (tf32-like: 11-bit mantissa, full-rate PE) by default;
set KMM_DT=f32 for full-precision fp32 matmuls (4x slower PE).
"""

import os
import sys

sys.path.insert(0, "/opt/trn_rl_repo")

import numpy as np

B, SEQ, DIM, NHEAD, DH = 2, 2048, 2048, 16, 128
HL = NHEAD // 8  # heads per core = 2
NCORES = 8
NT = B * SEQ  # 4096 flattened rows
SCALE = 1.0 / np.sqrt(DH)

_PROG = {}


def _round_f32r(a):
    """Round fp32 array to fp32r (tf32-like): 8-bit exp, 11-bit stored mantissa,
    low 12 bits zero. Round-to-nearest-even."""
    b = np.ascontiguousarray(a, dtype=np.float32).view(np.uint32).astype(np.uint64)
    r = ((b + 0x7FF + ((b >> 12) & 1)) & np.uint64(0xFFFFF000)).astype(np.uint32)
    return r.view(np.float32)


def _build(mm_f32r: bool):
    import concourse.tile as tile
    from concourse import bacc, mybir

    f32 = mybir.dt.float32
    f32r = mybir.dt.float32r
    Exp = mybir.ActivationFunctionType.Exp
    dmm = f32r if mm_f32r else f32

    nc = bacc.Bacc("TRN2", target_bir_lowering=False, debug=False, num_devices=NCORES)

    xt_d = nc.dram_tensor("xt", [DIM, NT], dmm, kind="ExternalInput")
    wqk_d = nc.dram_tensor("wqk", [DIM, 4 * DH], dmm, kind="ExternalInput")
    wv_d = nc.dram_tensor("wv", [DIM, HL * DH], dmm, kind="ExternalInput")
    wo_d = nc.dram_tensor("wo_r", [HL * DH, DIM], dmm, kind="ExternalInput")
    cos_d = nc.dram_tensor("cos2", [128, 32 * 128], dmm, kind="ExternalInput")
    sin_d = nc.dram_tensor("sin2", [128, 32 * 128], dmm, kind="ExternalInput")
    ident_d = nc.dram_tensor("ident", [128, 128], dmm, kind="ExternalInput")
    ones_d = nc.dram_tensor("ones", [128, 1], dmm, kind="ExternalInput")
    onesrow_d = nc.dram_tensor("onesrow", [1, 128], dmm, kind="ExternalInput")
    out_d = nc.dram_tensor("out_p", [NT, DIM], f32, kind="ExternalOutput")

    vn_d = nc.dram_tensor("v_n", [NT, HL * DH], dmm)

    with tile.TileContext(nc) as tc:
        with (
            nc.allow_low_precision(reason="fp32r (tf32-like) matmul pipeline"),
            tc.tile_pool(name="const", bufs=1) as cp,
        ):
            ident = cp.tile([128, 128], dmm)
            ones = cp.tile([128, 1], dmm)
            onesrow = cp.tile([1, 128], dmm)
            # q^T / k^T SBUF-resident across phases: [tensor t][128 dh, NT]
            qkt_res = [
                cp.tile([128, NT], dmm, name=f"qktres{t}", tag=f"qktres{t}")
                for t in range(4)
            ]

            # ---------------- Phase A: qkv projection + rope + transpose ----
            ctx_bi = tc.tile_pool(name="bin", bufs=2)
            bi = ctx_bi.__enter__()
            bload = {}

            def load_bj(b, j):
                v_sb = bi.tile([128, 16, DH], dmm, tag="v", name=f"vsb{b}{j}")
                nc.sync.dma_start(
                    v_sb,
                    vn_d[b * SEQ : (b + 1) * SEQ, j * DH : (j + 1) * DH].rearrange(
                        "(c p) d -> p c d", p=128
                    ),
                )
                bload[(b, j)] = (
                    qkt_res[2 + j][:, b * SEQ : (b + 1) * SEQ],
                    qkt_res[j][:, b * SEQ : (b + 1) * SEQ],
                    v_sb,
                )

            with (
                tc.tile_pool(name="aconst", bufs=1) as ac,
                tc.tile_pool(name="axs", bufs=2) as axs,
                tc.tile_pool(name="awork", bufs=3) as aw,
                tc.tile_pool(name="apsum", bufs=2, space="PSUM") as aps,
                tc.tile_pool(name="atps", bufs=2, space="PSUM") as atp,
            ):
                wqk_sb = ac.tile([128, 16, 4 * DH], dmm)
                wqk_src = wqk_d.rearrange("(c p) m -> p c m", p=128)
                wv_sb = ac.tile([128, 16, HL * DH], dmm)
                wv_src = wv_d.rearrange("(c p) m -> p c m", p=128)
                xt_all = xt_d.rearrange("(c p) n -> p c n", p=128)
                SLW = 256  # n-slice width
                xs0 = axs.tile([128, 16, SLW], dmm, tag="xs", bufs=4)
                for cc in range(16):
                    nc.sync.dma_start(wqk_sb[:, cc, :], wqk_src[:, cc, :])
                    nc.sync.dma_start(xs0[:, cc, :], xt_all[:, cc, 0:SLW])
                    nc.sync.dma_start(wv_sb[:, cc, :], wv_src[:, cc, :])
                xs1 = axs.tile([128, 16, SLW], dmm, tag="xs", bufs=4)
                for cg in range(4):
                    nc.sync.dma_start(
                        xs1[:, 4 * cg : 4 * cg + 4, :],
                        xt_all[:, 4 * cg : 4 * cg + 4, SLW : 2 * SLW],
                    )
                nc.sync.dma_start(ident, ident_d[:, :])
                nc.sync.dma_start(ones, ones_d[:, :])
                nc.sync.dma_start(onesrow, onesrow_d[:, :])

                for sl in range(NT // SLW):
                    n0 = sl * SLW
                    if sl == 0:
                        xs = xs0
                    elif sl == 1:
                        xs = xs1
                    else:
                        xs = axs.tile([128, 16, SLW], dmm, tag="xs", bufs=4)
                        xt_src = xt_all[:, :, n0 : n0 + SLW]
                        for cg in range(4):
                            nc.sync.dma_start(
                                xs[:, 4 * cg : 4 * cg + 4, :], xt_src[:, 4 * cg : 4 * cg + 4, :]
                            )
                    cos_sb = aw.tile([128, 2, 128], dmm, tag="cos", bufs=2)
                    nc.sync.dma_start(
                        cos_sb.rearrange("p i f -> p (i f)"), cos_d[:, n0 : n0 + SLW]
                    )
                    sin_sb = aw.tile([128, 2, 128], dmm, tag="sin", bufs=2)
                    nc.sync.dma_start(
                        sin_sb.rearrange("p i f -> p (i f)"), sin_d[:, n0 : n0 + SLW]
                    )
                    for st in range(SLW // 128):
                        i = sl * (SLW // 128) + st
                        g0 = n0 + st * 128
                        xsl = xs[:, :, st * 128 : (st + 1) * 128]
                        qkps = aps.tile([128, 4 * DH], f32, tag="qk", bufs=3)
                        for cc in range(16):
                            nc.tensor.matmul(
                                qkps,
                                xsl[:, cc, :],
                                wqk_sb[:, cc, :],
                                start=(cc == 0),
                                stop=(cc == 15),
                            )
                        vps = aps.tile([128, HL * DH], f32, tag="v", bufs=3)
                        for cc in range(16):
                            nc.tensor.matmul(
                                vps,
                                xsl[:, cc, :],
                                wv_sb[:, cc, :],
                                start=(cc == 0),
                                stop=(cc == 15),
                            )
                        vsb = aw.tile([128, HL * DH], dmm, tag="vsb", bufs=2)
                        nc.scalar.copy(vsb, vps)
                        nc.sync.dma_start(vn_d[g0 : g0 + 128, :], vsb)

                        # rope: cols [q0 q1 k0 k1], each 128 = [64 even | 64 odd]
                        rt = aw.tile([128, 4 * DH], dmm, tag="rt", bufs=2)
                        ca = cos_sb[:, st, :].rearrange("p (t f) -> p t f", t=2)
                        sa = sin_sb[:, st, :].rearrange("p (t f) -> p t f", t=2)
                        for g in range(2):
                            blk = qkps[:, g * 256 : (g + 1) * 256].rearrange(
                                "p (t h f) -> p t h f", t=2, h=2
                            )
                            rbl = rt[:, g * 256 : (g + 1) * 256].rearrange(
                                "p (t h f) -> p t h f", t=2, h=2
                            )
                            ev, od = blk[:, :, 0, :], blk[:, :, 1, :]
                            tA = aw.tile([128, 2, 64], f32, tag="tA", bufs=1)
                            tB = aw.tile([128, 2, 64], f32, tag="tB", bufs=1)
                            nc.vector.tensor_mul(tA, od, sa)
                            nc.vector.tensor_mul(tB, ev, ca)
                            nc.vector.tensor_sub(rbl[:, :, 0, :], tB, tA)
                            tC = aw.tile([128, 2, 64], f32, tag="tC", bufs=1)
                            tD = aw.tile([128, 2, 64], f32, tag="tD", bufs=1)
                            nc.vector.tensor_mul(tC, ev, sa)
                            nc.vector.tensor_mul(tD, od, ca)
                            nc.vector.tensor_add(rbl[:, :, 1, :], tD, tC)

                        for t in range(4):
                            tp = atp.tile([128, 128], dmm, tag="tp")
                            nc.tensor.transpose(tp, rt[:, t * 128 : (t + 1) * 128], ident)
                            nc.scalar.copy(qkt_res[t][:, g0 : g0 + 128], tp)
                    if sl == 7:
                        load_bj(0, 0)
                        load_bj(0, 1)

            # ---------- Phase B+C: attention + output projection ------------
            with (
                tc.tile_pool(name="bprobs", bufs=1) as bp,
                tc.tile_pool(name="bwork", bufs=3) as bw,
                tc.tile_pool(name="bavres", bufs=3) as bav_sb,
                tc.tile_pool(name="cot", bufs=2) as cot,
                tc.tile_pool(name="bs", bufs=2, space="PSUM") as bs,
                tc.tile_pool(name="bsum", bufs=1, space="PSUM") as bsm,
                tc.tile_pool(name="brb", bufs=1, space="PSUM") as brb,
                tc.tile_pool(name="bav", bufs=2, space="PSUM") as bav,
            ):
                wo_sb = bp.tile([128, HL, DIM], dmm, tag="wo", bufs=1)
                nc.sync.dma_start(wo_sb, wo_d.rearrange("(j p) o -> p j o", p=128))
                avres = {}

                def emit_b_qtile(b, j, qt_i, kt_sb, qt_sb, v_sb, av_r):
                    q0 = qt_i * 512
                    probs = bp.tile([128, 16, 512], dmm, tag="probs", name="probs")
                    for kp in range(8):
                        sps = bs.tile([128, 2, 512], f32, tag="s", name="sps")
                        for u in range(2):
                            kt_i = 2 * kp + u
                            nc.tensor.matmul(
                                sps[:, u, :],
                                kt_sb[:, kt_i * 128 : (kt_i + 1) * 128],
                                qt_sb[:, q0 : q0 + 512],
                                start=True,
                                stop=True,
                            )
                        nc.scalar.activation(probs[:, 2 * kp : 2 * kp + 2, :], sps, Exp)
                    sum_ps = bsm.tile([1, 512], f32, tag="sum", name="sum_ps")
                    for kt_i in range(16):
                        nc.tensor.matmul(
                            sum_ps,
                            ones,
                            probs[:, kt_i, :],
                            start=(kt_i == 0),
                            stop=(kt_i == 15),
                        )
                    sum_sb = bw.tile([1, 512], dmm, tag="sumsb", name="sum_sb")
                    nc.vector.tensor_copy(sum_sb, sum_ps)
                    rbc_ps = brb.tile([128, 512], f32, tag="rbc", name="rbc_ps")
                    nc.tensor.matmul(rbc_ps, onesrow, sum_sb, start=True, stop=True)
                    rbc = bw.tile([128, 512], f32, tag="rbcsb", name="rbc")
                    nc.vector.reciprocal_approx_fast(rbc, rbc_ps)
                    avps = bav.tile([128, 512], f32, tag="av", name="avps")
                    for cc in range(16):
                        nc.tensor.matmul(
                            avps,
                            v_sb[:, cc, :],
                            probs[:, cc, :],
                            start=(cc == 0),
                            stop=(cc == 15),
                        )
                    nc.vector.tensor_mul(av_r[:, q0 : q0 + 512], avps, rbc)

                def emit_c_subtile(b, nl, tail):
                    g0 = b * SEQ + nl * 128
                    ot = cot.tile([128, DIM], f32, tag="ot", name="ot")
                    for do in range(4):
                        ops = bav.tile([128, 512], f32, tag="av", name="ops")
                        for j in range(HL):
                            nc.tensor.matmul(
                                ops,
                                avres[(b, j)][:, nl * 128 : (nl + 1) * 128],
                                wo_sb[:, j, do * 512 : (do + 1) * 512],
                                start=(j == 0),
                                stop=(j == 1),
                            )
                        osl = ot[:, do * 512 : (do + 1) * 512]
                        if tail:
                            nc.scalar.copy(osl, ops)
                            nc.sync.dma_start(
                                out_d[g0 : g0 + 128, do * 512 : (do + 1) * 512], osl
                            )
                        else:
                            nc.vector.tensor_copy(osl, ops)
                    if not tail:
                        nc.sync.dma_start(out_d[g0 : g0 + 128, :], ot)

                # attention per (batch, head); C(b) interleaved into B(b, j=1):
                # C(b, 4*qt..4*qt+3) reads exactly the av columns written by
                # qtile qt of both heads, so it can follow immediately.
                for b in range(B):
                    for j in range(HL):
                        if (b, j) not in bload:
                            load_bj(b, j)
                        kt_sb, qt_sb, v_sb = bload.pop((b, j))
                        av_r = bav_sb.tile([128, SEQ], dmm, tag="avres", name="av_r")
                        avres[(b, j)] = av_r
                        for qt_i in range(4):
                            emit_b_qtile(b, j, qt_i, kt_sb, qt_sb, v_sb, av_r)
                            if j == 1:
                                tail = b == 1 and qt_i == 3
                                for u in range(4):
                                    emit_c_subtile(b, 4 * qt_i + u, tail=tail)

            ctx_bi.__exit__(None, None, None)

    nc.compile()
    return nc


def _get_prog():
    mm_f32r = os.environ.get("KMM_DT", "f32r") == "f32r"
    key = ("prog", mm_f32r)
    if key not in _PROG:
        _PROG[key] = _build(mm_f32r)
    return _PROG[key], mm_f32r


def _shard(x, freqs_cis, wqkv, wo, mm_f32r):
    rnd = _round_f32r if mm_f32r else (lambda a: np.ascontiguousarray(a, np.float32))
    x = np.asarray(x, dtype=np.float32)
    freqs_cis = np.asarray(freqs_cis, dtype=np.float32)
    wqkv = np.asarray(wqkv, dtype=np.float32)
    wo = np.asarray(wo, dtype=np.float32)

    xt = rnd(x.reshape(NT, DIM).T)

    cos = freqs_cis[:, :, 0]  # [SEQ, 64]
    sin = freqs_cis[:, :, 1]
    cosb = np.concatenate([cos] * B, axis=0)  # [NT, 64], row n = b*SEQ + pos
    sinb = np.concatenate([sin] * B, axis=0)
    cos2n = np.concatenate([cosb, cosb], axis=1)  # [NT, 128] dup halves
    sin2n = np.concatenate([sinb, sinb], axis=1)
    # partition-major for contiguous DMA: [128 p, 32 i, 128 f] flattened
    cos2 = rnd(cos2n.reshape(32, 128, 128).transpose(1, 0, 2).reshape(128, 32 * 128))
    sin2 = rnd(sin2n.reshape(32, 128, 128).transpose(1, 0, 2).reshape(128, 32 * 128))

    perm = np.concatenate([np.arange(0, DH, 2), np.arange(1, DH, 2)])  # de-interleave
    consts = {
        "ident": np.eye(128, dtype=np.float32),
        "ones": np.ones((128, 1), np.float32),
        "onesrow": np.ones((1, 128), np.float32),
    }
    in_maps = []
    for c in range(NCORES):
        h0 = c * HL
        wq = [wqkv[:, h * DH : (h + 1) * DH][:, perm] * SCALE for h in (h0, h0 + 1)]
        wk = [wqkv[:, DIM + h * DH : DIM + (h + 1) * DH][:, perm] for h in (h0, h0 + 1)]
        wqk_c = rnd(np.concatenate(wq + wk, axis=1))  # [DIM, 512]
        wv_c = rnd(wqkv[:, 2 * DIM + h0 * DH : 2 * DIM + (h0 + HL) * DH])  # [DIM, 256]
        wo_c = rnd(wo[h0 * DH : (h0 + HL) * DH, :])  # [256, DIM]
        in_maps.append(
            {
                "xt": xt,
                "wqk": wqk_c,
                "wv": wv_c,
                "wo_r": wo_c,
                "cos2": cos2,
                "sin2": sin2,
                **consts,
            }
        )
    return in_maps


def _run(in_maps, trace=False, **kw):
    from concourse.bass_utils import run_bass_kernel_spmd

    prog, _ = _get_prog()
    return run_bass_kernel_spmd(prog, in_maps, list(range(NCORES)), trace=trace, **kw)


def kernel(x, freqs_cis, wqkv, wo):
    _, mm_f32r = _get_prog()
    in_maps = _shard(x, freqs_cis, wqkv, wo, mm_f32r)
    res = _run(in_maps, trace=False)
    acc = np.zeros((NT, DIM), dtype=np.float32)
    for c in range(NCORES):
        acc += res.results[c]["out_p"]
    return acc.reshape(B, SEQ, DIM)



# revision 6
# speedup vs baseline: 1.2080x; 1.2080x over previous
"""Trainium2 Bass kernel for nn_BidirectionalAttention (B=2, N=2048, D=2048, H=16).

Head-parallel tensor sharding across 8 NeuronCores (2 heads/core), bf16
matmul pipeline (fp32 PSUM accumulation):

  phase A: qkv projection from x^T (x chunks stationary, w moving), rope on
           natural layout via DVE, PE-transpose q,k into [head_dim, seq]
           (transposes delayed one tile so PE never waits on DVE rope);
           v stays SBUF-resident in natural [seq, dh] layout with a ones
           column appended per head.
  phase B: per (batch, head): transposed scores s^T[k,q] = k^T.T @ q^T,
           exp on ScalarE -> probs (bf16). AV uses probs pieces as the
           STATIONARY operand against rhs [v | ones] so each [128q x 129]
           psum accumulates both attn@v (natural layout) and the softmax
           denominator in column 128 -- no separate ones-matmul sum pass.
           Scale by fast reciprocal (per-partition scalar broadcast), then
           PE-transpose av into [dh, seq] for the output projection.
  phase C: output projection partial = av^T.T @ wo_rows, interleaved with
           phase B per batch; partials DMA'd out in bf16.
Host: shard/transpose/bf16-round inputs, sum the 8 partial outputs in fp32
(the "all-reduce after wo" done at gather time).
"""

import os
import sys

sys.path.insert(0, "/opt/trn_rl_repo")

import numpy as np
import ml_dtypes

B, SEQ, DIM, NHEAD, DH = 2, 2048, 2048, 16, 128
HL = NHEAD // 8  # heads per core = 2
NCORES = 8
NT = B * SEQ  # 4096 flattened rows
SCALE = 1.0 / np.sqrt(DH)

_PROG = {}


def _build():
    import concourse.tile as tile
    from concourse import bacc, mybir

    f32 = mybir.dt.float32
    bf = mybir.dt.bfloat16
    Exp = mybir.ActivationFunctionType.Exp

    nc = bacc.Bacc("TRN2", target_bir_lowering=False, debug=False, num_devices=NCORES)

    xt_d = nc.dram_tensor("xt", [DIM, NT], bf, kind="ExternalInput")
    wqk_d = nc.dram_tensor("wqk", [DIM, 4 * DH], bf, kind="ExternalInput")
    wv_d = nc.dram_tensor("wv", [DIM, HL * DH], bf, kind="ExternalInput")
    wo_d = nc.dram_tensor("wo_r", [HL * DH, DIM], bf, kind="ExternalInput")
    cos_d = nc.dram_tensor("cos2", [128, 32 * 128], bf, kind="ExternalInput")
    sin_d = nc.dram_tensor("sin2", [128, 32 * 128], bf, kind="ExternalInput")
    ident_d = nc.dram_tensor("ident", [128, 128], bf, kind="ExternalInput")
    out_d = nc.dram_tensor("out_p", [NT, DIM], bf, kind="ExternalOutput")

    with tile.TileContext(nc) as tc:
        with (
            nc.allow_low_precision(reason="bf16 matmul pipeline, fp32 accumulation"),
            tc.tile_pool(name="const", bufs=1) as cp,
        ):
            ident = cp.tile([128, 128], bf)
            # q^T / k^T SBUF-resident: [tensor t][128 dh, NT]  (q0 q1 k0 k1)
            qkt_res = [
                cp.tile([128, NT], bf, name=f"qktres{t}", tag=f"qktres{t}")
                for t in range(4)
            ]
            # v natural layout, SBUF-resident: [128 k, 32 rowchunk, 2 head, 129]
            # col 128 of each head slot is the ones column for the denominator.
            v_all = cp.tile([128, 32, HL, 129], bf)
            # av^T per (b, j): [128 dh, SEQ]
            avres = {
                (b, j): cp.tile([128, SEQ], bf, name=f"avres{b}{j}", tag=f"avres{b}{j}")
                for b in range(B)
                for j in range(HL)
            }
            wqk_sb = cp.tile([128, 16, 4 * DH], bf)
            wv_sb = cp.tile([128, 16, HL * DH], bf)
            wo_sb = cp.tile([128, HL, DIM], bf)

            nc.sync.dma_start(ident, ident_d[:, :])
            nc.vector.memset(v_all[:, :, :, 128:129], 1.0)

            # ---------------- Phase A: qkv projection + rope + transpose ----
            with (
                tc.tile_pool(name="axs", bufs=3) as axs,
                tc.tile_pool(name="awork", bufs=2) as aw,
                tc.tile_pool(name="apsum", bufs=2, space="PSUM") as aps,
                tc.tile_pool(name="atps", bufs=2, space="PSUM") as atp,
            ):
                wqk_src = wqk_d.rearrange("(c p) m -> p c m", p=128)
                wv_src = wv_d.rearrange("(c p) m -> p c m", p=128)
                xt_all = xt_d.rearrange("(c p) n -> p c n", p=128)
                SLW = 256  # n-slice width (2 row-tiles)
                xs0 = axs.tile([128, 16, SLW], bf, tag="xs", bufs=3)
                for cc in range(16):
                    nc.sync.dma_start(wqk_sb[:, cc, :], wqk_src[:, cc, :])
                    nc.sync.dma_start(xs0[:, cc, :], xt_all[:, cc, 0:SLW])
                    nc.sync.dma_start(wv_sb[:, cc, :], wv_src[:, cc, :])
                xs1 = axs.tile([128, 16, SLW], bf, tag="xs", bufs=3)
                for cg in range(4):
                    nc.sync.dma_start(
                        xs1[:, 4 * cg : 4 * cg + 4, :],
                        xt_all[:, 4 * cg : 4 * cg + 4, SLW : 2 * SLW],
                    )
                nc.sync.dma_start(wo_sb, wo_d.rearrange("(j p) o -> p j o", p=128))

                pend = None  # (rt tile, g0) awaiting transpose; delayed 1 tile

                def emit_transposes(rt, g0):
                    tp = atp.tile([128, 4, 128], bf, tag="tp")
                    for t in range(4):
                        nc.tensor.transpose(
                            tp[:, t, :], rt[:, t * 128 : (t + 1) * 128], ident
                        )
                    for t in range(4):
                        nc.scalar.copy(qkt_res[t][:, g0 : g0 + 128], tp[:, t, :])

                for sl in range(NT // SLW):
                    n0 = sl * SLW
                    if sl == 0:
                        xs = xs0
                    elif sl == 1:
                        xs = xs1
                    else:
                        xs = axs.tile([128, 16, SLW], bf, tag="xs", bufs=3)
                        xt_src = xt_all[:, :, n0 : n0 + SLW]
                        for cg in range(4):
                            nc.sync.dma_start(
                                xs[:, 4 * cg : 4 * cg + 4, :],
                                xt_src[:, 4 * cg : 4 * cg + 4, :],
                            )
                    cos_sb = aw.tile([128, 2, 128], bf, tag="cos", bufs=2)
                    nc.sync.dma_start(
                        cos_sb.rearrange("p i f -> p (i f)"), cos_d[:, n0 : n0 + SLW]
                    )
                    sin_sb = aw.tile([128, 2, 128], bf, tag="sin", bufs=2)
                    nc.sync.dma_start(
                        sin_sb.rearrange("p i f -> p (i f)"), sin_d[:, n0 : n0 + SLW]
                    )
                    for st in range(SLW // 128):
                        i = sl * (SLW // 128) + st
                        g0 = n0 + st * 128
                        xsl = xs[:, :, st * 128 : (st + 1) * 128]
                        qkps = aps.tile([128, 4 * DH], f32, tag="qk", bufs=3)
                        for cc in range(16):
                            nc.tensor.matmul(
                                qkps,
                                xsl[:, cc, :],
                                wqk_sb[:, cc, :],
                                start=(cc == 0),
                                stop=(cc == 15),
                            )
                        vps = aps.tile([128, HL * DH], f32, tag="v", bufs=2)
                        for cc in range(16):
                            nc.tensor.matmul(
                                vps,
                                xsl[:, cc, :],
                                wv_sb[:, cc, :],
                                start=(cc == 0),
                                stop=(cc == 15),
                            )
                        if pend is not None:
                            emit_transposes(*pend)
                        # v -> v_all natural layout (per-head slots, cols 0:128)
                        nc.scalar.copy(
                            v_all[:, i, :, 0:128],
                            vps.rearrange("p (j d) -> p j d", j=HL),
                        )

                        # rope: cols [q0 q1 k0 k1], each 128 = [64 even | 64 odd]
                        rt = aw.tile([128, 4 * DH], bf, tag="rt", bufs=2)
                        ca = cos_sb[:, st, :].rearrange("p (t f) -> p t f", t=2)
                        sa = sin_sb[:, st, :].rearrange("p (t f) -> p t f", t=2)
                        for g in range(2):
                            blk = qkps[:, g * 256 : (g + 1) * 256].rearrange(
                                "p (t h f) -> p t h f", t=2, h=2
                            )
                            rbl = rt[:, g * 256 : (g + 1) * 256].rearrange(
                                "p (t h f) -> p t h f", t=2, h=2
                            )
                            ev, od = blk[:, :, 0, :], blk[:, :, 1, :]
                            tA = aw.tile([128, 2, 64], bf, tag="tA", bufs=1)
                            tB = aw.tile([128, 2, 64], bf, tag="tB", bufs=1)
                            nc.vector.tensor_mul(tA, od, sa)
                            nc.vector.tensor_mul(tB, ev, ca)
                            nc.vector.tensor_sub(rbl[:, :, 0, :], tB, tA)
                            tC = aw.tile([128, 2, 64], bf, tag="tC", bufs=1)
                            tD = aw.tile([128, 2, 64], bf, tag="tD", bufs=1)
                            nc.vector.tensor_mul(tC, ev, sa)
                            nc.vector.tensor_mul(tD, od, ca)
                            nc.vector.tensor_add(rbl[:, :, 1, :], tD, tC)
                        pend = (rt, g0)
                emit_transposes(*pend)

            # ---------- Phase B+C: attention + output projection ------------
            with (
                tc.tile_pool(name="bprobs", bufs=1) as bp,
                tc.tile_pool(name="bwork", bufs=2) as bw,
                tc.tile_pool(name="cot", bufs=2) as cot,
                tc.tile_pool(name="bs", bufs=2, space="PSUM") as bs,
                tc.tile_pool(name="bav", bufs=3, space="PSUM") as bav,
                tc.tile_pool(name="batp", bufs=1, space="PSUM") as batp,
            ):
                probs_of = {}
                pend_drain = [None]  # (b, j, qt, [av_s x4]) awaiting transpose

                def emit_scores(b, j, qt):
                    """scores + exp for one 512-wide q group: fills probs."""
                    kt_sb = qkt_res[2 + j][:, b * SEQ : (b + 1) * SEQ]
                    qt_sb = qkt_res[j][:, b * SEQ : (b + 1) * SEQ]
                    q0 = qt * 512
                    probs = bp.tile([128, 16, 512], bf, tag="probs", bufs=2, name="probs")
                    probs_of[(b, j, qt)] = probs
                    for kp in range(8):
                        sps = bs.tile([128, 2, 512], f32, tag="s", name="sps")
                        for u in range(2):
                            kt_i = 2 * kp + u
                            nc.tensor.matmul(
                                sps[:, u, :],
                                kt_sb[:, kt_i * 128 : (kt_i + 1) * 128],
                                qt_sb[:, q0 : q0 + 512],
                                start=True,
                                stop=True,
                            )
                        nc.scalar.activation(probs[:, 2 * kp : 2 * kp + 2, :], sps, Exp)

                def flush_drain():
                    """PE-transpose the previous q-group's scaled av into avres;
                    for j==1 groups, follow with the output-projection subtiles
                    that consume exactly those avres columns (both heads done)."""
                    if pend_drain[0] is None:
                        return
                    b, j, qt, av_ss = pend_drain[0]
                    pend_drain[0] = None
                    av_r = avres[(b, j)]
                    avT = batp.tile([128, 4, 128], bf, tag="avT", name="avT")
                    for qs in range(4):
                        nc.tensor.transpose(avT[:, qs, :], av_ss[qs], ident)
                        nc.vector.tensor_copy(
                            av_r[:, qt * 512 + qs * 128 : qt * 512 + (qs + 1) * 128],
                            avT[:, qs, :],
                        )
                    if j == 1:
                        tail = b == 1 and qt == 3
                        for u in range(4):
                            emit_c_subtile(b, 4 * qt + u, tail=tail)

                def emit_av(b, j, qt):
                    """AV + denominator for one 512-wide q group (drain deferred)."""
                    probs = probs_of.pop((b, j, qt))
                    avps = []
                    for qsp in range(2):  # pairs of 128-wide q subtiles
                        avp = bav.tile([128, 2, 256], f32, tag="avp", name="avp")
                        avps.append(avp)
                        for u in range(2):
                            qs = 2 * qsp + u
                            for kc in range(16):
                                nc.tensor.matmul(
                                    avp[:, u, 0:129],
                                    probs[:, kc, qs * 128 : (qs + 1) * 128],
                                    v_all[:, b * 16 + kc, j, :],
                                    start=(kc == 0),
                                    stop=(kc == 15),
                                )
                    # scales emitted BEFORE flush_drain so the avp-ring WAR
                    # deps of the C-subtile psum grabs see these readers.
                    av_ss = []
                    for qs in range(4):
                        avp, u = avps[qs // 2], qs % 2
                        rcp = bw.tile([128, 1], f32, tag="rcp", bufs=2, name="rcp")
                        nc.vector.reciprocal_approx_fast(rcp, avp[:, u, 128:129])
                        av_s = bw.tile([128, 128], bf, tag="avs", bufs=8, name="avs")
                        nc.vector.tensor_scalar_mul(av_s, avp[:, u, 0:128], rcp)
                        av_ss.append(av_s)
                    flush_drain()
                    pend_drain[0] = (b, j, qt, av_ss)

                def emit_c_subtile(b, nl, tail):
                    g0 = b * SEQ + nl * 128
                    ot = cot.tile([128, DIM], bf, tag="ot", name="ot")
                    for do in range(4):
                        ops = bav.tile([128, 512], f32, tag="avp", bufs=3, name="ops")
                        for j in range(HL):
                            nc.tensor.matmul(
                                ops,
                                avres[(b, j)][:, nl * 128 : (nl + 1) * 128],
                                wo_sb[:, j, do * 512 : (do + 1) * 512],
                                start=(j == 0),
                                stop=(j == 1),
                            )
                        osl = ot[:, do * 512 : (do + 1) * 512]
                        if tail:
                            nc.scalar.copy(osl, ops)
                            nc.sync.dma_start(
                                out_d[g0 : g0 + 128, do * 512 : (do + 1) * 512], osl
                            )
                        elif do % 2 == 0:
                            nc.vector.tensor_copy(osl, ops)
                        else:
                            nc.scalar.copy(osl, ops)
                    if not tail:
                        nc.sync.dma_start(out_d[g0 : g0 + 128, :], ot)

                # attention per (batch, head): scores pipelined one q-group
                # ahead of AV, av-transpose drain (and the j==1 C subtiles)
                # deferred one q-group further so PE never waits on DVE.
                for b in range(B):
                    for j in range(HL):
                        emit_scores(b, j, 0)
                        for qt in range(4):
                            if qt < 3:
                                emit_scores(b, j, qt + 1)
                            emit_av(b, j, qt)
                flush_drain()

    nc.compile()
    return nc


def _get_prog():
    if "prog" not in _PROG:
        _PROG["prog"] = _build()
    return _PROG["prog"], False


def _shard(x, freqs_cis, wqkv, wo, mm_f32r=False):
    bf = ml_dtypes.bfloat16
    x = np.asarray(x, dtype=np.float32)
    freqs_cis = np.asarray(freqs_cis, dtype=np.float32)
    wqkv = np.asarray(wqkv, dtype=np.float32)
    wo = np.asarray(wo, dtype=np.float32)

    xt = np.ascontiguousarray(x.reshape(NT, DIM).T).astype(bf)

    cos = freqs_cis[:, :, 0]  # [SEQ, 64]
    sin = freqs_cis[:, :, 1]
    cosb = np.concatenate([cos] * B, axis=0)  # [NT, 64], row n = b*SEQ + pos
    sinb = np.concatenate([sin] * B, axis=0)
    cos2n = np.concatenate([cosb, cosb], axis=1)  # [NT, 128] dup halves
    sin2n = np.concatenate([sinb, sinb], axis=1)
    # partition-major for contiguous DMA: [128 p, 32 i, 128 f] flattened
    cos2 = cos2n.reshape(32, 128, 128).transpose(1, 0, 2).reshape(128, 32 * 128)
    sin2 = sin2n.reshape(32, 128, 128).transpose(1, 0, 2).reshape(128, 32 * 128)
    cos2 = np.ascontiguousarray(cos2).astype(bf)
    sin2 = np.ascontiguousarray(sin2).astype(bf)

    perm = np.concatenate([np.arange(0, DH, 2), np.arange(1, DH, 2)])  # de-interleave
    ident = np.eye(128, dtype=np.float32).astype(bf)
    in_maps = []
    for c in range(NCORES):
        h0 = c * HL
        wq = [wqkv[:, h * DH : (h + 1) * DH][:, perm] * SCALE for h in (h0, h0 + 1)]
        wk = [wqkv[:, DIM + h * DH : DIM + (h + 1) * DH][:, perm] for h in (h0, h0 + 1)]
        wqk_c = np.concatenate(wq + wk, axis=1).astype(bf)  # [DIM, 512]
        wv_c = np.ascontiguousarray(
            wqkv[:, 2 * DIM + h0 * DH : 2 * DIM + (h0 + HL) * DH]
        ).astype(bf)  # [DIM, 256]
        wo_c = np.ascontiguousarray(wo[h0 * DH : (h0 + HL) * DH, :]).astype(bf)
        in_maps.append(
            {
                "xt": xt,
                "wqk": wqk_c,
                "wv": wv_c,
                "wo_r": wo_c,
                "cos2": cos2,
                "sin2": sin2,
                "ident": ident,
            }
        )
    return in_maps


def _run(in_maps, trace=False, **kw):
    from concourse.bass_utils import run_bass_kernel_spmd

    prog, _ = _get_prog()
    return run_bass_kernel_spmd(prog, in_maps, list(range(NCORES)), trace=trace, **kw)


def kernel(x, freqs_cis, wqkv, wo):
    _get_prog()
    in_maps = _shard(x, freqs_cis, wqkv, wo)
    res = _run(in_maps, trace=False)
    acc = np.zeros((NT, DIM), dtype=np.float32)
    for c in range(NCORES):
        acc += np.asarray(res.results[c]["out_p"], dtype=np.float32)
    return acc.reshape(B, SEQ, DIM)


# revision 13
# speedup vs baseline: 1.2607x; 1.0436x over previous
"""Trainium2 Bass kernel for nn_BidirectionalAttention (B=2, N=2048, D=2048, H=16).

Head-parallel tensor sharding across 8 NeuronCores (2 heads/core), bf16
matmul pipeline (fp32 PSUM accumulation):

  phase A: qkv projection from x^T (x chunks stationary, w moving), rope on
           natural layout via DVE, PE-transpose q,k into [head_dim, seq]
           (transposes delayed one tile so PE never waits on DVE rope);
           v stays SBUF-resident in natural [seq, dh] layout with a ones
           column appended per head.
  phase B: per (batch, head): transposed scores s^T[k,q] = k^T.T @ q^T,
           exp on ScalarE -> probs (bf16). AV uses probs pieces as the
           STATIONARY operand against rhs [v | ones] so each [128q x 129]
           psum accumulates both attn@v (natural layout) and the softmax
           denominator in column 128 -- no separate ones-matmul sum pass.
           Scale by fast reciprocal (per-partition scalar broadcast), then
           PE-transpose av into [dh, seq] for the output projection.
  phase C: output projection partial = av^T.T @ wo_rows, interleaved with
           phase B per batch; partials DMA'd out in bf16.
Host: shard/transpose/bf16-round inputs, sum the 8 partial outputs in fp32
(the "all-reduce after wo" done at gather time).
"""

import os
import sys

sys.path.insert(0, "/opt/trn_rl_repo")

import numpy as np
import ml_dtypes

B, SEQ, DIM, NHEAD, DH = 2, 2048, 2048, 16, 128
HL = NHEAD // 8  # heads per core = 2
NCORES = 8
NT = B * SEQ  # 4096 flattened rows
SCALE = 1.0 / np.sqrt(DH)

_PROG = {}


def _build():
    import concourse.tile as tile
    from concourse import bacc, mybir

    f32 = mybir.dt.float32
    bf = mybir.dt.bfloat16
    Exp = mybir.ActivationFunctionType.Exp

    nc = bacc.Bacc("TRN2", target_bir_lowering=False, debug=False, num_devices=NCORES)

    xt_d = nc.dram_tensor("xt", [DIM, NT], bf, kind="ExternalInput")
    wqk_d = nc.dram_tensor("wqk", [DIM, 4 * DH], bf, kind="ExternalInput")
    wv_d = nc.dram_tensor("wv", [DIM, HL * DH], bf, kind="ExternalInput")
    wo_d = nc.dram_tensor("wo_r", [HL * DH, DIM], bf, kind="ExternalInput")
    cos_d = nc.dram_tensor("cos2", [128, 32 * 128], bf, kind="ExternalInput")
    sin_d = nc.dram_tensor("sin2", [128, 32 * 128], bf, kind="ExternalInput")
    ident_d = nc.dram_tensor("ident", [128, 128], bf, kind="ExternalInput")
    out_d = nc.dram_tensor("out_p", [NT, DIM], bf, kind="ExternalOutput")

    with tile.TileContext(nc) as tc:
        with (
            nc.allow_low_precision(reason="bf16 matmul pipeline, fp32 accumulation"),
            tc.tile_pool(name="const", bufs=1) as cp,
        ):
            ident = cp.tile([128, 128], bf)
            # q^T / k^T SBUF-resident: [tensor t][128 dh, NT]  (q0 q1 k0 k1)
            qkt_res = [
                cp.tile([128, NT], bf, name=f"qktres{t}", tag=f"qktres{t}")
                for t in range(4)
            ]
            # v natural layout, SBUF-resident: [128 k, 32 rowchunk, 2 head, 129]
            # col 128 of each head slot is the ones column for the denominator.
            v_all = cp.tile([128, 32, HL, 129], bf)
            # av^T per (b, j): [128 dh, SEQ]
            avres = {
                (b, j): cp.tile([128, SEQ], bf, name=f"avres{b}{j}", tag=f"avres{b}{j}")
                for b in range(B)
                for j in range(HL)
            }
            wqk_sb = cp.tile([128, 16, 4 * DH], bf)
            wv_sb = cp.tile([128, 16, HL * DH], bf)
            wo_sb = cp.tile([128, HL, DIM], bf)
            cos_all = cp.tile([128, 32, 128], bf)
            sin_all = cp.tile([128, 32, 128], bf)

            nc.sync.dma_start(ident, ident_d[:, :])
            nc.vector.memset(v_all[:, :, :, 128:129], 1.0)

            # ---------------- Phase A: qkv projection + rope + transpose ----
            with (
                tc.tile_pool(name="axs", bufs=3) as axs,
                tc.tile_pool(name="awork", bufs=2) as aw,
                tc.tile_pool(name="apsum", bufs=2, space="PSUM") as aps,
                tc.tile_pool(name="atps", bufs=2, space="PSUM") as atp,
            ):
                wqk_src = wqk_d.rearrange("(c p) m -> p c m", p=128)
                wv_src = wv_d.rearrange("(c p) m -> p c m", p=128)
                xt_all = xt_d.rearrange("(c p) n -> p c n", p=128)
                SLW = 256  # n-slice width (2 row-tiles)
                # batched DMA issues: the sync queue costs ~600ns per issue,
                # so keep the issue count low and the first-needed bytes first.
                xs0 = axs.tile([128, 16, SLW], bf, tag="xs", bufs=3)
                for cg in range(4):
                    nc.sync.dma_start(
                        xs0[:, 4 * cg : 4 * cg + 4, :],
                        xt_all[:, 4 * cg : 4 * cg + 4, 0:SLW],
                    )
                for half in range(2):
                    nc.sync.dma_start(
                        wqk_sb[:, 8 * half : 8 * half + 8, :],
                        wqk_src[:, 8 * half : 8 * half + 8, :],
                    )
                nc.sync.dma_start(wv_sb, wv_src)
                xs1 = axs.tile([128, 16, SLW], bf, tag="xs", bufs=3)
                nc.sync.dma_start(xs1, xt_all[:, :, SLW : 2 * SLW])
                nc.sync.dma_start(
                    cos_all.rearrange("p i f -> p (i f)"), cos_d[:, :]
                )
                nc.sync.dma_start(
                    sin_all.rearrange("p i f -> p (i f)"), sin_d[:, :]
                )
                nc.sync.dma_start(wo_sb, wo_d.rearrange("(j p) o -> p j o", p=128))

                pend = None  # (rt tile, g0) awaiting transpose; delayed 1 tile

                def emit_transposes(rt, g0):
                    tp = atp.tile([128, 4, 128], bf, tag="tp")
                    for t in range(4):
                        nc.tensor.transpose(
                            tp[:, t, :], rt[:, t * 128 : (t + 1) * 128], ident
                        )
                    for t in range(4):
                        nc.scalar.copy(qkt_res[t][:, g0 : g0 + 128], tp[:, t, :])

                xs_tiles = {0: xs0, 1: xs1}
                for sl in range(NT // SLW):
                    n0 = sl * SLW
                    xs = xs_tiles.pop(sl)
                    if sl + 2 < NT // SLW:  # prefetch one slice ahead
                        nxt = axs.tile([128, 16, SLW], bf, tag="xs", bufs=3)
                        nc.sync.dma_start(
                            nxt, xt_all[:, :, (sl + 2) * SLW : (sl + 3) * SLW]
                        )
                        xs_tiles[sl + 2] = nxt
                    for st in range(SLW // 128):
                        i = sl * (SLW // 128) + st
                        g0 = n0 + st * 128
                        xsl = xs[:, :, st * 128 : (st + 1) * 128]
                        qkps = aps.tile([128, 4 * DH], f32, tag="qk", bufs=3)
                        for cc in range(16):
                            nc.tensor.matmul(
                                qkps,
                                xsl[:, cc, :],
                                wqk_sb[:, cc, :],
                                start=(cc == 0),
                                stop=(cc == 15),
                            )
                        vps = aps.tile([128, HL * DH], f32, tag="v", bufs=2)
                        for cc in range(16):
                            nc.tensor.matmul(
                                vps,
                                xsl[:, cc, :],
                                wv_sb[:, cc, :],
                                start=(cc == 0),
                                stop=(cc == 15),
                            )
                        if pend is not None:
                            emit_transposes(*pend)
                        # v -> v_all natural layout (per-head slots, cols 0:128)
                        nc.scalar.copy(
                            v_all[:, i, :, 0:128],
                            vps.rearrange("p (j d) -> p j d", j=HL),
                        )

                        # rope: cols [q0 q1 k0 k1], each 128 = [64 even | 64 odd]
                        rt = aw.tile([128, 4 * DH], bf, tag="rt", bufs=2)
                        ca = cos_all[:, i, :].rearrange("p (t f) -> p t f", t=2)
                        sa = sin_all[:, i, :].rearrange("p (t f) -> p t f", t=2)
                        for g in range(2):
                            blk = qkps[:, g * 256 : (g + 1) * 256].rearrange(
                                "p (t h f) -> p t h f", t=2, h=2
                            )
                            rbl = rt[:, g * 256 : (g + 1) * 256].rearrange(
                                "p (t h f) -> p t h f", t=2, h=2
                            )
                            ev, od = blk[:, :, 0, :], blk[:, :, 1, :]
                            tA = aw.tile([128, 2, 64], bf, tag="tA", bufs=1)
                            tB = aw.tile([128, 2, 64], bf, tag="tB", bufs=1)
                            nc.vector.tensor_mul(tA, od, sa)
                            nc.vector.tensor_mul(tB, ev, ca)
                            nc.vector.tensor_sub(rbl[:, :, 0, :], tB, tA)
                            tC = aw.tile([128, 2, 64], bf, tag="tC", bufs=1)
                            tD = aw.tile([128, 2, 64], bf, tag="tD", bufs=1)
                            nc.vector.tensor_mul(tC, ev, sa)
                            nc.vector.tensor_mul(tD, od, ca)
                            nc.vector.tensor_add(rbl[:, :, 1, :], tD, tC)
                        pend = (rt, g0)
                emit_transposes(*pend)

            # ---------- Phase B+C: attention + output projection ------------
            with (
                tc.tile_pool(name="bprobs", bufs=1) as bp,
                tc.tile_pool(name="bwork", bufs=2) as bw,
                tc.tile_pool(name="cot", bufs=2) as cot,
                tc.tile_pool(name="bs", bufs=2, space="PSUM") as bs,
                tc.tile_pool(name="bav", bufs=3, space="PSUM") as bav,
                tc.tile_pool(name="batp", bufs=1, space="PSUM") as batp,
            ):
                probs_of = {}
                pend_drain = [None]  # (b, j, qt, [av_s x4]) awaiting transpose

                def emit_scores(b, j, qt):
                    """scores + exp for one 512-wide q group: fills probs."""
                    kt_sb = qkt_res[2 + j][:, b * SEQ : (b + 1) * SEQ]
                    qt_sb = qkt_res[j][:, b * SEQ : (b + 1) * SEQ]
                    q0 = qt * 512
                    probs = bp.tile([128, 16, 512], bf, tag="probs", bufs=3, name="probs")
                    probs_of[(b, j, qt)] = probs
                    for kp in range(8):
                        sps = bs.tile([128, 2, 512], f32, tag="s", name="sps")
                        for u in range(2):
                            kt_i = 2 * kp + u
                            nc.tensor.matmul(
                                sps[:, u, :],
                                kt_sb[:, kt_i * 128 : (kt_i + 1) * 128],
                                qt_sb[:, q0 : q0 + 512],
                                start=True,
                                stop=True,
                            )
                        nc.scalar.activation(probs[:, 2 * kp : 2 * kp + 2, :], sps, Exp)

                def flush_drain():
                    """PE-transpose the previous q-group's scaled av into avres;
                    for j==1 groups, follow with the output-projection subtiles
                    that consume exactly those avres columns (both heads done)."""
                    if pend_drain[0] is None:
                        return
                    b, j, qt, av_ss = pend_drain[0]
                    pend_drain[0] = None
                    av_r = avres[(b, j)]
                    avT = batp.tile([128, 4, 128], bf, tag="avT", name="avT")
                    for qs in range(4):
                        nc.tensor.transpose(avT[:, qs, :], av_ss[qs], ident)
                        nc.vector.tensor_copy(
                            av_r[:, qt * 512 + qs * 128 : qt * 512 + (qs + 1) * 128],
                            avT[:, qs, :],
                        )
                    if j == 1:
                        tail = b == 1 and qt == 3
                        for u in range(4):
                            emit_c_subtile(b, 4 * qt + u, tail=tail)

                def emit_av(b, j, qt):
                    """AV + denominator for one 512-wide q group (drain deferred)."""
                    probs = probs_of.pop((b, j, qt))
                    avps = []
                    for qsp in range(2):  # pairs of 128-wide q subtiles
                        avp = bav.tile([128, 2, 256], f32, tag="avp", name="avp")
                        avps.append(avp)
                        for u in range(2):
                            qs = 2 * qsp + u
                            for kc in range(16):
                                nc.tensor.matmul(
                                    avp[:, u, 0:129],
                                    probs[:, kc, qs * 128 : (qs + 1) * 128],
                                    v_all[:, b * 16 + kc, j, :],
                                    start=(kc == 0),
                                    stop=(kc == 15),
                                )
                    # scales emitted BEFORE flush_drain so the avp-ring WAR
                    # deps of the C-subtile psum grabs see these readers.
                    av_ss = []
                    for qs in range(4):
                        avp, u = avps[qs // 2], qs % 2
                        rcp = bw.tile([128, 1], f32, tag="rcp", bufs=2, name="rcp")
                        nc.vector.reciprocal_approx_fast(rcp, avp[:, u, 128:129])
                        av_s = bw.tile([128, 128], bf, tag="avs", bufs=8, name="avs")
                        nc.vector.tensor_scalar_mul(av_s, avp[:, u, 0:128], rcp)
                        av_ss.append(av_s)
                    flush_drain()
                    pend_drain[0] = (b, j, qt, av_ss)

                def emit_c_subtile(b, nl, tail):
                    g0 = b * SEQ + nl * 128
                    ot = cot.tile([128, DIM], bf, tag="ot", bufs=4, name="ot")
                    for do in range(4):
                        ops = bav.tile([128, 512], f32, tag="avp", bufs=3, name="ops")
                        for j in range(HL):
                            nc.tensor.matmul(
                                ops,
                                avres[(b, j)][:, nl * 128 : (nl + 1) * 128],
                                wo_sb[:, j, do * 512 : (do + 1) * 512],
                                start=(j == 0),
                                stop=(j == 1),
                            )
                        osl = ot[:, do * 512 : (do + 1) * 512]
                        if do % 2 == 0:
                            nc.vector.tensor_copy(osl, ops)
                        else:
                            nc.scalar.copy(osl, ops)
                        if tail:
                            nc.sync.dma_start(
                                out_d[g0 : g0 + 128, do * 512 : (do + 1) * 512], osl
                            )
                    if not tail:
                        nc.sync.dma_start(out_d[g0 : g0 + 128, :], ot)

                # attention with the two heads interleaved per q-group so the
                # ScalarE exp stream (the per-head critical resource) overlaps
                # the other head's PE work; scores pipelined one q-group ahead
                # of AV; av-transpose drains (and the C subtiles, which need
                # both heads) deferred one group further so PE never waits.
                for b in range(B):
                    emit_scores(b, 0, 0)
                    emit_scores(b, 1, 0)
                    for qt in range(4):
                        if qt < 3:
                            emit_scores(b, 0, qt + 1)
                        emit_av(b, 0, qt)
                        if qt < 3:
                            emit_scores(b, 1, qt + 1)
                        emit_av(b, 1, qt)
                flush_drain()

    nc.compile()
    return nc


def _get_prog():
    if "prog" not in _PROG:
        _PROG["prog"] = _build()
    return _PROG["prog"], False


def _shard(x, freqs_cis, wqkv, wo, mm_f32r=False):
    bf = ml_dtypes.bfloat16
    x = np.asarray(x, dtype=np.float32)
    freqs_cis = np.asarray(freqs_cis, dtype=np.float32)
    wqkv = np.asarray(wqkv, dtype=np.float32)
    wo = np.asarray(wo, dtype=np.float32)

    xt = np.ascontiguousarray(x.reshape(NT, DIM).T).astype(bf)

    cos = freqs_cis[:, :, 0]  # [SEQ, 64]
    sin = freqs_cis[:, :, 1]
    cosb = np.concatenate([cos] * B, axis=0)  # [NT, 64], row n = b*SEQ + pos
    sinb = np.concatenate([sin] * B, axis=0)
    cos2n = np.concatenate([cosb, cosb], axis=1)  # [NT, 128] dup halves
    sin2n = np.concatenate([sinb, sinb], axis=1)
    # partition-major for contiguous DMA: [128 p, 32 i, 128 f] flattened
    cos2 = cos2n.reshape(32, 128, 128).transpose(1, 0, 2).reshape(128, 32 * 128)
    sin2 = sin2n.reshape(32, 128, 128).transpose(1, 0, 2).reshape(128, 32 * 128)
    cos2 = np.ascontiguousarray(cos2).astype(bf)
    sin2 = np.ascontiguousarray(sin2).astype(bf)

    perm = np.concatenate([np.arange(0, DH, 2), np.arange(1, DH, 2)])  # de-interleave
    ident = np.eye(128, dtype=np.float32).astype(bf)
    in_maps = []
    for c in range(NCORES):
        h0 = c * HL
        wq = [wqkv[:, h * DH : (h + 1) * DH][:, perm] * SCALE for h in (h0, h0 + 1)]
        wk = [wqkv[:, DIM + h * DH : DIM + (h + 1) * DH][:, perm] for h in (h0, h0 + 1)]
        wqk_c = np.concatenate(wq + wk, axis=1).astype(bf)  # [DIM, 512]
        wv_c = np.ascontiguousarray(
            wqkv[:, 2 * DIM + h0 * DH : 2 * DIM + (h0 + HL) * DH]
        ).astype(bf)  # [DIM, 256]
        wo_c = np.ascontiguousarray(wo[h0 * DH : (h0 + HL) * DH, :]).astype(bf)
        in_maps.append(
            {
                "xt": xt,
                "wqk": wqk_c,
                "wv": wv_c,
                "wo_r": wo_c,
                "cos2": cos2,
                "sin2": sin2,
                "ident": ident,
            }
        )
    return in_maps


def _run(in_maps, trace=False, **kw):
    from concourse.bass_utils import run_bass_kernel_spmd

    prog, _ = _get_prog()
    return run_bass_kernel_spmd(prog, in_maps, list(range(NCORES)), trace=trace, **kw)


def kernel(x, freqs_cis, wqkv, wo):
    _get_prog()
    in_maps = _shard(x, freqs_cis, wqkv, wo)
    res = _run(in_maps, trace=False)
    acc = np.zeros((NT, DIM), dtype=np.float32)
    for c in range(NCORES):
        acc += np.asarray(res.results[c]["out_p"], dtype=np.float32)
    return acc.reshape(B, SEQ, DIM)


# revision 19
# speedup vs baseline: 1.2788x; 1.0144x over previous
"""Trainium2 Bass kernel for nn_BidirectionalAttention (B=2, N=2048, D=2048, H=16).

Head-parallel tensor sharding across 8 NeuronCores (2 heads/core), bf16
matmul pipeline (fp32 PSUM accumulation):

  phase A: qkv projection from x^T (x chunks stationary, w moving), rope on
           natural layout via DVE, PE-transpose q,k into [head_dim, seq]
           (transposes delayed one tile so PE never waits on DVE rope);
           v stays SBUF-resident in natural [seq, dh] layout with a ones
           column appended per head.
  phase B: per (batch, head): transposed scores s^T[k,q] = k^T.T @ q^T,
           exp on ScalarE -> probs (bf16). AV uses probs pieces as the
           STATIONARY operand against rhs [v | ones] so each [128q x 129]
           psum accumulates both attn@v (natural layout) and the softmax
           denominator in column 128 -- no separate ones-matmul sum pass.
           Scale by fast reciprocal (per-partition scalar broadcast), then
           PE-transpose av into [dh, seq] for the output projection.
  phase C: output projection partial = av^T.T @ wo_rows, interleaved with
           phase B per batch; partials DMA'd out in bf16.
Host: shard/transpose/bf16-round inputs, sum the 8 partial outputs in fp32
(the "all-reduce after wo" done at gather time).
"""

import os
import sys

sys.path.insert(0, "/opt/trn_rl_repo")

import numpy as np
import ml_dtypes

B, SEQ, DIM, NHEAD, DH = 2, 2048, 2048, 16, 128
HL = NHEAD // 8  # heads per core = 2
NCORES = 8
NT = B * SEQ  # 4096 flattened rows
SCALE = 1.0 / np.sqrt(DH)

_PROG = {}


def _build():
    import concourse.tile as tile
    from concourse import bacc, mybir

    f32 = mybir.dt.float32
    bf = mybir.dt.bfloat16
    Exp = mybir.ActivationFunctionType.Exp

    nc = bacc.Bacc("TRN2", target_bir_lowering=False, debug=False, num_devices=NCORES)

    # all inputs host-pre-tiled into partition-major layouts so every DMA is
    # ~128 long contiguous descriptors (descriptor generation on the sync
    # queue costs ~2.7ns/descriptor and was the startup bottleneck).
    xtt_d = nc.dram_tensor("xtt", [32, 128, 16, 128], bf, kind="ExternalInput")
    wqk_d = nc.dram_tensor("wqkt", [128, 16, 4 * DH], bf, kind="ExternalInput")
    wv_d = nc.dram_tensor("wvt", [128, 16, HL * DH], bf, kind="ExternalInput")
    wo_d = nc.dram_tensor("wot", [128, HL, DIM], bf, kind="ExternalInput")
    cos_d = nc.dram_tensor("cos2", [128, 32 * 128], bf, kind="ExternalInput")
    sin_d = nc.dram_tensor("sin2", [128, 32 * 128], bf, kind="ExternalInput")
    ident_d = nc.dram_tensor("ident", [128, 128], bf, kind="ExternalInput")
    out_d = nc.dram_tensor("out_p", [NT, DIM], bf, kind="ExternalOutput")

    with tile.TileContext(nc) as tc:
        with (
            nc.allow_low_precision(reason="bf16 matmul pipeline, fp32 accumulation"),
            tc.tile_pool(name="const", bufs=1) as cp,
        ):
            ident = cp.tile([128, 128], bf)
            # q^T / k^T SBUF-resident: [tensor t][128 dh, NT]  (q0 q1 k0 k1)
            qkt_res = [
                cp.tile([128, NT], bf, name=f"qktres{t}", tag=f"qktres{t}")
                for t in range(4)
            ]
            # v natural layout, SBUF-resident: [128 k, 32 rowchunk, 2 head, 129]
            # col 128 of each head slot is the ones column for the denominator.
            v_all = cp.tile([128, 32, HL, 129], bf)
            # av^T per (b, j): [128 dh, SEQ]
            avres = {
                (b, j): cp.tile([128, SEQ], bf, name=f"avres{b}{j}", tag=f"avres{b}{j}")
                for b in range(B)
                for j in range(HL)
            }
            wqk_sb = cp.tile([128, 16, 4 * DH], bf)
            wv_sb = cp.tile([128, 16, HL * DH], bf)
            wo_sb = cp.tile([128, HL, DIM], bf)
            cos_all = cp.tile([128, 32, 128], bf)
            sin_all = cp.tile([128, 32, 128], bf)

            nc.sync.dma_start(ident, ident_d[:, :])
            nc.vector.memset(v_all[:, :, :, 128:129], 1.0)

            # ---------------- Phase A: qkv projection + rope + transpose ----
            with (
                tc.tile_pool(name="axs", bufs=3) as axs,
                tc.tile_pool(name="awork", bufs=2) as aw,
                tc.tile_pool(name="apsum", bufs=2, space="PSUM") as aps,
                tc.tile_pool(name="atps", bufs=2, space="PSUM") as atp,
            ):
                xs_tiles = {}

                def fetch(i):
                    t = axs.tile([128, 16, 128], bf, tag="xs", bufs=4, name="xs")
                    nc.sync.dma_start(t, xtt_d[i])
                    xs_tiles[i] = t

                # first-needed bytes first: x tile 0 and wqk quarters pace the
                # first row-tile's accumulation chain.
                fetch(0)
                for q in range(4):
                    nc.sync.dma_start(
                        wqk_sb[:, 4 * q : 4 * q + 4, :], wqk_d[:, 4 * q : 4 * q + 4, :]
                    )
                fetch(1)
                nc.sync.dma_start(wv_sb, wv_d[:, :, :])
                fetch(2)
                fetch(3)
                nc.sync.dma_start(cos_all.rearrange("p i f -> p (i f)"), cos_d[:, :])
                nc.sync.dma_start(sin_all.rearrange("p i f -> p (i f)"), sin_d[:, :])
                nc.sync.dma_start(wo_sb, wo_d[:, :, :])

                pend = None  # (rt tile, g0) awaiting transpose; delayed 1 tile

                def emit_transposes(rt, g0):
                    tp = atp.tile([128, 4, 128], bf, tag="tp")
                    for t in range(4):
                        nc.tensor.transpose(
                            tp[:, t, :], rt[:, t * 128 : (t + 1) * 128], ident
                        )
                    for t in range(4):
                        nc.scalar.copy(qkt_res[t][:, g0 : g0 + 128], tp[:, t, :])

                for i in range(32):
                    g0 = i * 128
                    xs = xs_tiles.pop(i)
                    if i + 4 < 32:  # prefetch, ring depth 4
                        fetch(i + 4)
                    qkps = aps.tile([128, 4 * DH], f32, tag="qk", bufs=3)
                    for cc in range(16):
                        nc.tensor.matmul(
                            qkps,
                            xs[:, cc, :],
                            wqk_sb[:, cc, :],
                            start=(cc == 0),
                            stop=(cc == 15),
                        )
                    vps = aps.tile([128, HL * DH], f32, tag="v", bufs=2)
                    for cc in range(16):
                        nc.tensor.matmul(
                            vps,
                            xs[:, cc, :],
                            wv_sb[:, cc, :],
                            start=(cc == 0),
                            stop=(cc == 15),
                        )
                    if pend is not None:
                        emit_transposes(*pend)
                    # v -> v_all natural layout (per-head slots, cols 0:128)
                    nc.scalar.copy(
                        v_all[:, i, :, 0:128],
                        vps.rearrange("p (j d) -> p j d", j=HL),
                    )

                    # rope: cols [q0 q1 k0 k1], each 128 = [64 even | 64 odd]
                    rt = aw.tile([128, 4 * DH], bf, tag="rt", bufs=2)
                    ca = cos_all[:, i, :].rearrange("p (t f) -> p t f", t=2)
                    sa = sin_all[:, i, :].rearrange("p (t f) -> p t f", t=2)
                    for g in range(2):
                        blk = qkps[:, g * 256 : (g + 1) * 256].rearrange(
                            "p (t h f) -> p t h f", t=2, h=2
                        )
                        rbl = rt[:, g * 256 : (g + 1) * 256].rearrange(
                            "p (t h f) -> p t h f", t=2, h=2
                        )
                        ev, od = blk[:, :, 0, :], blk[:, :, 1, :]
                        tA = aw.tile([128, 2, 64], bf, tag="tA", bufs=1)
                        tB = aw.tile([128, 2, 64], bf, tag="tB", bufs=1)
                        nc.vector.tensor_mul(tA, od, sa)
                        nc.vector.tensor_mul(tB, ev, ca)
                        nc.vector.tensor_sub(rbl[:, :, 0, :], tB, tA)
                        tC = aw.tile([128, 2, 64], bf, tag="tC", bufs=1)
                        tD = aw.tile([128, 2, 64], bf, tag="tD", bufs=1)
                        nc.vector.tensor_mul(tC, ev, sa)
                        nc.vector.tensor_mul(tD, od, ca)
                        nc.vector.tensor_add(rbl[:, :, 1, :], tD, tC)
                    pend = (rt, g0)
                emit_transposes(*pend)

            # ---------- Phase B+C: attention + output projection ------------
            with (
                tc.tile_pool(name="bprobs", bufs=1) as bp,
                tc.tile_pool(name="bwork", bufs=2) as bw,
                tc.tile_pool(name="cot", bufs=2) as cot,
                tc.tile_pool(name="bs", bufs=2, space="PSUM") as bs,
                tc.tile_pool(name="bav", bufs=3, space="PSUM") as bav,
                tc.tile_pool(name="batp", bufs=1, space="PSUM") as batp,
            ):
                probs_of = {}
                pend_drain = [None]  # (b, j, qt, [av_s x4]) awaiting transpose

                def emit_scores(b, j, qt):
                    """scores + exp for one 512-wide q group: fills probs."""
                    kt_sb = qkt_res[2 + j][:, b * SEQ : (b + 1) * SEQ]
                    qt_sb = qkt_res[j][:, b * SEQ : (b + 1) * SEQ]
                    q0 = qt * 512
                    probs = bp.tile([128, 16, 512], bf, tag="probs", bufs=3, name="probs")
                    probs_of[(b, j, qt)] = probs
                    for kp in range(8):
                        sps = bs.tile([128, 2, 512], f32, tag="s", name="sps")
                        for u in range(2):
                            kt_i = 2 * kp + u
                            nc.tensor.matmul(
                                sps[:, u, :],
                                kt_sb[:, kt_i * 128 : (kt_i + 1) * 128],
                                qt_sb[:, q0 : q0 + 512],
                                start=True,
                                stop=True,
                            )
                        nc.scalar.activation(probs[:, 2 * kp : 2 * kp + 2, :], sps, Exp)

                def flush_drain():
                    """PE-transpose the previous q-group's scaled av into avres;
                    for j==1 groups, follow with the output-projection subtiles
                    that consume exactly those avres columns (both heads done)."""
                    if pend_drain[0] is None:
                        return
                    b, j, qt, av_ss = pend_drain[0]
                    pend_drain[0] = None
                    av_r = avres[(b, j)]
                    avT = batp.tile([128, 4, 128], bf, tag="avT", name="avT")
                    tail = b == 1 and qt == 3
                    for qs in range(4):
                        nc.tensor.transpose(avT[:, qs, :], av_ss[qs], ident)
                        nc.vector.tensor_copy(
                            av_r[:, qt * 512 + qs * 128 : qt * 512 + (qs + 1) * 128],
                            avT[:, qs, :],
                        )
                        if j == 1:
                            emit_c_subtile(b, 4 * qt + qs, tail=tail)

                def emit_av(b, j, qt):
                    """AV + denominator for one 512-wide q group (drain deferred)."""
                    probs = probs_of.pop((b, j, qt))
                    avps = []
                    for qsp in range(2):  # pairs of 128-wide q subtiles
                        avp = bav.tile([128, 2, 256], f32, tag="avp", name="avp")
                        avps.append(avp)
                        for u in range(2):
                            qs = 2 * qsp + u
                            for kc in range(16):
                                nc.tensor.matmul(
                                    avp[:, u, 0:129],
                                    probs[:, kc, qs * 128 : (qs + 1) * 128],
                                    v_all[:, b * 16 + kc, j, :],
                                    start=(kc == 0),
                                    stop=(kc == 15),
                                )
                    # scales emitted BEFORE flush_drain so the avp-ring WAR
                    # deps of the C-subtile psum grabs see these readers.
                    av_ss = []
                    for qs in range(4):
                        avp, u = avps[qs // 2], qs % 2
                        rcp = bw.tile([128, 1], f32, tag="rcp", bufs=2, name="rcp")
                        nc.vector.reciprocal_approx_fast(rcp, avp[:, u, 128:129])
                        av_s = bw.tile([128, 128], bf, tag="avs", bufs=8, name="avs")
                        nc.vector.tensor_scalar_mul(av_s, avp[:, u, 0:128], rcp)
                        av_ss.append(av_s)
                    flush_drain()
                    pend_drain[0] = (b, j, qt, av_ss)

                def emit_c_subtile(b, nl, tail):
                    g0 = b * SEQ + nl * 128
                    ot = cot.tile([128, DIM], bf, tag="ot", bufs=4, name="ot")
                    for do in range(4):
                        ops = bav.tile([128, 512], f32, tag="avp", bufs=3, name="ops")
                        for j in range(HL):
                            nc.tensor.matmul(
                                ops,
                                avres[(b, j)][:, nl * 128 : (nl + 1) * 128],
                                wo_sb[:, j, do * 512 : (do + 1) * 512],
                                start=(j == 0),
                                stop=(j == 1),
                            )
                        osl = ot[:, do * 512 : (do + 1) * 512]
                        if do % 2 == 0:
                            nc.vector.tensor_copy(osl, ops)
                        else:
                            nc.scalar.copy(osl, ops)
                        if tail:
                            nc.sync.dma_start(
                                out_d[g0 : g0 + 128, do * 512 : (do + 1) * 512], osl
                            )
                    if not tail:
                        nc.sync.dma_start(out_d[g0 : g0 + 128, :], ot)

                # attention with the two heads interleaved per q-group so the
                # ScalarE exp stream (the per-head critical resource) overlaps
                # the other head's PE work; scores pipelined one q-group ahead
                # of AV; av-transpose drains (and the C subtiles, which need
                # both heads) deferred one group further so PE never waits.
                for b in range(B):
                    emit_scores(b, 0, 0)
                    emit_scores(b, 1, 0)
                    for qt in range(4):
                        if qt < 3:
                            emit_scores(b, 0, qt + 1)
                        emit_av(b, 0, qt)
                        if qt < 3:
                            emit_scores(b, 1, qt + 1)
                        emit_av(b, 1, qt)
                flush_drain()

    nc.compile()
    return nc


def _get_prog():
    if "prog" not in _PROG:
        _PROG["prog"] = _build()
    return _PROG["prog"], False


def _shard(x, freqs_cis, wqkv, wo, mm_f32r=False):
    bf = ml_dtypes.bfloat16
    x = np.asarray(x, dtype=np.float32)
    freqs_cis = np.asarray(freqs_cis, dtype=np.float32)
    wqkv = np.asarray(wqkv, dtype=np.float32)
    wo = np.asarray(wo, dtype=np.float32)

    # x^T pre-tiled per 128-row output tile: [32 tile, 128 p, 16 chunk, 128]
    xt = x.reshape(NT, DIM).T  # [DIM, NT]
    xtt = np.ascontiguousarray(
        xt.reshape(16, 128, 32, 128).transpose(2, 1, 0, 3)
    ).astype(bf)

    cos = freqs_cis[:, :, 0]  # [SEQ, 64]
    sin = freqs_cis[:, :, 1]
    cosb = np.concatenate([cos] * B, axis=0)  # [NT, 64], row n = b*SEQ + pos
    sinb = np.concatenate([sin] * B, axis=0)
    cos2n = np.concatenate([cosb, cosb], axis=1)  # [NT, 128] dup halves
    sin2n = np.concatenate([sinb, sinb], axis=1)
    # partition-major for contiguous DMA: [128 p, 32 i, 128 f] flattened
    cos2 = cos2n.reshape(32, 128, 128).transpose(1, 0, 2).reshape(128, 32 * 128)
    sin2 = sin2n.reshape(32, 128, 128).transpose(1, 0, 2).reshape(128, 32 * 128)
    cos2 = np.ascontiguousarray(cos2).astype(bf)
    sin2 = np.ascontiguousarray(sin2).astype(bf)

    perm = np.concatenate([np.arange(0, DH, 2), np.arange(1, DH, 2)])  # de-interleave
    ident = np.eye(128, dtype=np.float32).astype(bf)
    in_maps = []
    def ptile(a, inner):  # [16*128, m] -> [128 p, 16 chunk, m]
        m = a.shape[1]
        return np.ascontiguousarray(
            a.reshape(inner, 128, m).transpose(1, 0, 2)
        ).astype(bf)

    for c in range(NCORES):
        h0 = c * HL
        wq = [wqkv[:, h * DH : (h + 1) * DH][:, perm] * SCALE for h in (h0, h0 + 1)]
        wk = [wqkv[:, DIM + h * DH : DIM + (h + 1) * DH][:, perm] for h in (h0, h0 + 1)]
        wqk_c = ptile(np.concatenate(wq + wk, axis=1), 16)  # [128, 16, 512]
        wv_c = ptile(wqkv[:, 2 * DIM + h0 * DH : 2 * DIM + (h0 + HL) * DH], 16)
        wo_c = ptile(wo[h0 * DH : (h0 + HL) * DH, :], HL)  # [128, 2, DIM]
        in_maps.append(
            {
                "xtt": xtt,
                "wqkt": wqk_c,
                "wvt": wv_c,
                "wot": wo_c,
                "cos2": cos2,
                "sin2": sin2,
                "ident": ident,
            }
        )
    return in_maps


def _run(in_maps, trace=False, **kw):
    from concourse.bass_utils import run_bass_kernel_spmd

    prog, _ = _get_prog()
    return run_bass_kernel_spmd(prog, in_maps, list(range(NCORES)), trace=trace, **kw)


def kernel(x, freqs_cis, wqkv, wo):
    _get_prog()
    in_maps = _shard(x, freqs_cis, wqkv, wo)
    res = _run(in_maps, trace=False)
    acc = np.zeros((NT, DIM), dtype=np.float32)
    for c in range(NCORES):
        acc += np.asarray(res.results[c]["out_p"], dtype=np.float32)
    return acc.reshape(B, SEQ, DIM)


# revision 21
# speedup vs baseline: 1.3195x; 1.0318x over previous
"""Trainium2 Bass kernel for nn_BidirectionalAttention (B=2, N=2048, D=2048, H=16).

Head-parallel tensor sharding across 8 NeuronCores (2 heads/core), bf16
matmul pipeline (fp32 PSUM accumulation):

  phase A: qkv projection from x^T (x chunks stationary, w moving), rope on
           natural layout via DVE, PE-transpose q,k into [head_dim, seq]
           (transposes delayed one tile so PE never waits on DVE rope);
           v stays SBUF-resident in natural [seq, dh] layout with a ones
           column appended per head.
  phase B: per (batch, head): transposed scores s^T[k,q] = k^T.T @ q^T,
           exp on ScalarE -> probs (bf16). AV uses probs pieces as the
           STATIONARY operand against rhs [v | ones] so each [128q x 129]
           psum accumulates both attn@v (natural layout) and the softmax
           denominator in column 128 -- no separate ones-matmul sum pass.
           Scale by fast reciprocal (per-partition scalar broadcast), then
           PE-transpose av into [dh, seq] for the output projection.
  phase C: output projection partial = av^T.T @ wo_rows, interleaved with
           phase B per batch; partials DMA'd out in bf16.
Host: shard/transpose/bf16-round inputs, sum the 8 partial outputs in fp32
(the "all-reduce after wo" done at gather time).
"""

import os
import sys

sys.path.insert(0, "/opt/trn_rl_repo")

import numpy as np
import ml_dtypes

B, SEQ, DIM, NHEAD, DH = 2, 2048, 2048, 16, 128
HL = NHEAD // 8  # heads per core = 2
NCORES = 8
NT = B * SEQ  # 4096 flattened rows
SCALE = 1.0 / np.sqrt(DH)

_PROG = {}


def _build():
    import concourse.tile as tile
    from concourse import bacc, mybir

    f32 = mybir.dt.float32
    bf = mybir.dt.bfloat16
    Exp = mybir.ActivationFunctionType.Exp

    nc = bacc.Bacc("TRN2", target_bir_lowering=False, debug=False, num_devices=NCORES)

    # all inputs host-pre-tiled into partition-major layouts so every DMA is
    # ~128 long contiguous descriptors (descriptor generation on the sync
    # queue costs ~2.7ns/descriptor and was the startup bottleneck).
    xtt_d = nc.dram_tensor("xtt", [32, 128, 16, 128], bf, kind="ExternalInput")
    wqk_d = nc.dram_tensor("wqkt", [128, 16, 4 * DH], bf, kind="ExternalInput")
    wv_d = nc.dram_tensor("wvt", [128, 16, HL * DH], bf, kind="ExternalInput")
    wo_d = nc.dram_tensor("wot", [128, HL, DIM], bf, kind="ExternalInput")
    cos_d = nc.dram_tensor("cos2", [128, 32 * 128], bf, kind="ExternalInput")
    sin_d = nc.dram_tensor("sin2", [128, 32 * 128], bf, kind="ExternalInput")
    ident_d = nc.dram_tensor("ident", [128, 128], bf, kind="ExternalInput")
    out_d = nc.dram_tensor("out_p", [NT, DIM], bf, kind="ExternalOutput")

    with tile.TileContext(nc) as tc:
        with (
            nc.allow_low_precision(reason="bf16 matmul pipeline, fp32 accumulation"),
            tc.tile_pool(name="const", bufs=1) as cp,
        ):
            ident = cp.tile([128, 128], bf)
            # q^T / k^T SBUF-resident: [tensor t][128 dh, NT]  (q0 q1 k0 k1)
            qkt_res = [
                cp.tile([128, NT], bf, name=f"qktres{t}", tag=f"qktres{t}")
                for t in range(4)
            ]
            # v natural layout, SBUF-resident: [128 k, 32 rowchunk, 2 head, 129]
            # col 128 of each head slot is the ones column for the denominator.
            v_all = cp.tile([128, 32, HL, 129], bf)
            # av^T per (b, j): [128 dh, SEQ]
            avres = {
                (b, j): cp.tile([128, SEQ], bf, name=f"avres{b}{j}", tag=f"avres{b}{j}")
                for b in range(B)
                for j in range(HL)
            }
            wqk_sb = cp.tile([128, 16, 4 * DH], bf)
            wv_sb = cp.tile([128, 16, HL * DH], bf)
            wo_sb = cp.tile([128, HL, DIM], bf)
            cos_all = cp.tile([128, 32, 128], bf)
            sin_all = cp.tile([128, 32, 128], bf)

            nc.sync.dma_start(ident, ident_d[:, :])
            nc.vector.memset(v_all[:, :, :, 128:129], 1.0)

            # ---------------- Phase A: qkv projection + rope + transpose ----
            with (
                tc.tile_pool(name="axs", bufs=3) as axs,
                tc.tile_pool(name="awork", bufs=2) as aw,
                tc.tile_pool(name="apsum", bufs=2, space="PSUM") as aps,
                tc.tile_pool(name="atps", bufs=2, space="PSUM") as atp,
            ):
                xs_tiles = {}

                def fetch(i):
                    t = axs.tile([128, 16, 128], bf, tag="xs", bufs=4, name="xs")
                    nc.sync.dma_start(t, xtt_d[i])
                    xs_tiles[i] = t

                # first-needed bytes first: x tile 0 and wqk quarters pace the
                # first row-tile's accumulation chain.
                fetch(0)
                for q in range(4):
                    nc.sync.dma_start(
                        wqk_sb[:, 4 * q : 4 * q + 4, :], wqk_d[:, 4 * q : 4 * q + 4, :]
                    )
                fetch(1)
                nc.sync.dma_start(wv_sb, wv_d[:, :, :])
                fetch(2)
                fetch(3)
                nc.sync.dma_start(cos_all.rearrange("p i f -> p (i f)"), cos_d[:, :])
                nc.sync.dma_start(sin_all.rearrange("p i f -> p (i f)"), sin_d[:, :])
                nc.sync.dma_start(wo_sb, wo_d[:, :, :])

                pend = None  # (rt tile, g0) awaiting transpose; delayed 1 tile

                def emit_transposes(rt, g0):
                    tp = atp.tile([128, 4, 128], bf, tag="tp")
                    for t in range(4):
                        nc.tensor.transpose(
                            tp[:, t, :], rt[:, t * 128 : (t + 1) * 128], ident
                        )
                    for t in range(4):
                        nc.scalar.copy(qkt_res[t][:, g0 : g0 + 128], tp[:, t, :])

                for i in range(32):
                    g0 = i * 128
                    xs = xs_tiles.pop(i)
                    if i + 4 < 32:  # prefetch, ring depth 4
                        fetch(i + 4)
                    qkps = aps.tile([128, 4 * DH], f32, tag="qk", bufs=3)
                    for cc in range(16):
                        nc.tensor.matmul(
                            qkps,
                            xs[:, cc, :],
                            wqk_sb[:, cc, :],
                            start=(cc == 0),
                            stop=(cc == 15),
                        )
                    vps = aps.tile([128, HL * DH], f32, tag="v", bufs=2)
                    for cc in range(16):
                        nc.tensor.matmul(
                            vps,
                            xs[:, cc, :],
                            wv_sb[:, cc, :],
                            start=(cc == 0),
                            stop=(cc == 15),
                        )
                    if pend is not None:
                        emit_transposes(*pend)
                    # v -> v_all natural layout (per-head slots, cols 0:128)
                    nc.scalar.copy(
                        v_all[:, i, :, 0:128],
                        vps.rearrange("p (j d) -> p j d", j=HL),
                    )

                    # rope: cols [q0 q1 k0 k1], each 128 = [64 even | 64 odd]
                    rt = aw.tile([128, 4 * DH], bf, tag="rt", bufs=2)
                    ca = cos_all[:, i, :].rearrange("p (t f) -> p t f", t=2)
                    sa = sin_all[:, i, :].rearrange("p (t f) -> p t f", t=2)
                    for g in range(2):
                        blk = qkps[:, g * 256 : (g + 1) * 256].rearrange(
                            "p (t h f) -> p t h f", t=2, h=2
                        )
                        rbl = rt[:, g * 256 : (g + 1) * 256].rearrange(
                            "p (t h f) -> p t h f", t=2, h=2
                        )
                        ev, od = blk[:, :, 0, :], blk[:, :, 1, :]
                        tA = aw.tile([128, 2, 64], bf, tag="tA", bufs=1)
                        tB = aw.tile([128, 2, 64], bf, tag="tB", bufs=1)
                        nc.vector.tensor_mul(tA, od, sa)
                        nc.vector.tensor_mul(tB, ev, ca)
                        nc.vector.tensor_sub(rbl[:, :, 0, :], tB, tA)
                        tC = aw.tile([128, 2, 64], bf, tag="tC", bufs=1)
                        tD = aw.tile([128, 2, 64], bf, tag="tD", bufs=1)
                        nc.vector.tensor_mul(tC, ev, sa)
                        nc.vector.tensor_mul(tD, od, ca)
                        nc.vector.tensor_add(rbl[:, :, 1, :], tD, tC)
                    pend = (rt, g0)
                emit_transposes(*pend)

            # ---------- Phase B+C: attention + output projection ------------
            with (
                tc.tile_pool(name="bprobs", bufs=1) as bp,
                tc.tile_pool(name="bwork", bufs=2) as bw,
                tc.tile_pool(name="cot", bufs=2) as cot,
                tc.tile_pool(name="bs", bufs=2, space="PSUM") as bs,
                tc.tile_pool(name="bav", bufs=3, space="PSUM") as bav,
                tc.tile_pool(name="batp", bufs=1, space="PSUM") as batp,
            ):
                probs_of = {}
                pend_drain = [None]  # (b, j, qt, [av_s x4]) awaiting transpose

                def emit_scores(b, j, qt):
                    """scores + exp for one 512-wide q group: fills probs."""
                    kt_sb = qkt_res[2 + j][:, b * SEQ : (b + 1) * SEQ]
                    qt_sb = qkt_res[j][:, b * SEQ : (b + 1) * SEQ]
                    q0 = qt * 512
                    probs = bp.tile([128, 16, 512], bf, tag="probs", bufs=3, name="probs")
                    probs_of[(b, j, qt)] = probs
                    for kp in range(8):
                        sps = bs.tile([128, 2, 512], f32, tag="s", name="sps")
                        for u in range(2):
                            kt_i = 2 * kp + u
                            nc.tensor.matmul(
                                sps[:, u, :],
                                kt_sb[:, kt_i * 128 : (kt_i + 1) * 128],
                                qt_sb[:, q0 : q0 + 512],
                                start=True,
                                stop=True,
                            )
                        nc.scalar.activation(probs[:, 2 * kp : 2 * kp + 2, :], sps, Exp)

                def flush_drain():
                    """PE-transpose the previous q-group's scaled av into avres;
                    for j==1 groups, follow with the output-projection subtiles
                    that consume exactly those avres columns (both heads done)."""
                    if pend_drain[0] is None:
                        return
                    b, j, qt, av_ss = pend_drain[0]
                    pend_drain[0] = None
                    av_r = avres[(b, j)]
                    avT = batp.tile([128, 4, 128], bf, tag="avT", name="avT")
                    tail = b == 1 and qt == 3
                    for qs in range(4):
                        nc.tensor.transpose(avT[:, qs, :], av_ss[qs], ident)
                        nc.vector.tensor_copy(
                            av_r[:, qt * 512 + qs * 128 : qt * 512 + (qs + 1) * 128],
                            avT[:, qs, :],
                        )
                        # tail: interleave C so the final out-DMAs start early;
                        # otherwise emit C after all drains so its matmuls
                        # never wait on the just-queued DVE avres copies.
                        if j == 1 and tail:
                            emit_c_subtile(b, 4 * qt + qs, tail=True)
                    if j == 1 and not tail:
                        for qs in range(4):
                            emit_c_subtile(b, 4 * qt + qs, tail=False)

                def emit_av(b, j, qt):
                    """AV + denominator for one 512-wide q group (drain deferred)."""
                    probs = probs_of.pop((b, j, qt))
                    avps = []
                    for qsp in range(2):  # pairs of 128-wide q subtiles
                        avp = bav.tile([128, 2, 256], f32, tag="avp", name="avp")
                        avps.append(avp)
                        for u in range(2):
                            qs = 2 * qsp + u
                            for kc in range(16):
                                nc.tensor.matmul(
                                    avp[:, u, 0:129],
                                    probs[:, kc, qs * 128 : (qs + 1) * 128],
                                    v_all[:, b * 16 + kc, j, :],
                                    start=(kc == 0),
                                    stop=(kc == 15),
                                )
                    # scales emitted BEFORE flush_drain so the avp-ring WAR
                    # deps of the C-subtile psum grabs see these readers.
                    av_ss = []
                    for qs in range(4):
                        avp, u = avps[qs // 2], qs % 2
                        rcp = bw.tile([128, 1], f32, tag="rcp", bufs=2, name="rcp")
                        nc.vector.reciprocal_approx_fast(rcp, avp[:, u, 128:129])
                        av_s = bw.tile([128, 128], bf, tag="avs", bufs=8, name="avs")
                        nc.vector.tensor_scalar_mul(av_s, avp[:, u, 0:128], rcp)
                        av_ss.append(av_s)
                    flush_drain()
                    pend_drain[0] = (b, j, qt, av_ss)

                def emit_c_subtile(b, nl, tail):
                    g0 = b * SEQ + nl * 128
                    ot = cot.tile([128, DIM], bf, tag="ot", bufs=4, name="ot")
                    for do in range(4):
                        ops = bav.tile([128, 512], f32, tag="avp", bufs=3, name="ops")
                        for j in range(HL):
                            nc.tensor.matmul(
                                ops,
                                avres[(b, j)][:, nl * 128 : (nl + 1) * 128],
                                wo_sb[:, j, do * 512 : (do + 1) * 512],
                                start=(j == 0),
                                stop=(j == 1),
                            )
                        osl = ot[:, do * 512 : (do + 1) * 512]
                        # ScalarE is the binding engine during phase B (exp
                        # stream), so PSUM->SBUF output copies go to DVE;
                        # at the tail ACT is idle, so alternate for speed.
                        if tail and do % 2 == 1:
                            nc.scalar.copy(osl, ops)
                        else:
                            nc.vector.tensor_copy(osl, ops)
                        if tail:
                            nc.sync.dma_start(
                                out_d[g0 : g0 + 128, do * 512 : (do + 1) * 512], osl
                            )
                    if not tail:
                        nc.sync.dma_start(out_d[g0 : g0 + 128, :], ot)

                # attention with the two heads interleaved per q-group so the
                # ScalarE exp stream (the per-head critical resource) overlaps
                # the other head's PE work; scores pipelined one q-group ahead
                # of AV; av-transpose drains (and the C subtiles, which need
                # both heads) deferred one group further so PE never waits.
                for b in range(B):
                    emit_scores(b, 0, 0)
                    emit_scores(b, 1, 0)
                    for qt in range(4):
                        if qt < 3:
                            emit_scores(b, 0, qt + 1)
                        emit_av(b, 0, qt)
                        if qt < 3:
                            emit_scores(b, 1, qt + 1)
                        emit_av(b, 1, qt)
                flush_drain()

    nc.compile()
    return nc


def _get_prog():
    if "prog" not in _PROG:
        _PROG["prog"] = _build()
    return _PROG["prog"], False


def _shard(x, freqs_cis, wqkv, wo, mm_f32r=False):
    bf = ml_dtypes.bfloat16
    x = np.asarray(x, dtype=np.float32)
    freqs_cis = np.asarray(freqs_cis, dtype=np.float32)
    wqkv = np.asarray(wqkv, dtype=np.float32)
    wo = np.asarray(wo, dtype=np.float32)

    # x^T pre-tiled per 128-row output tile: [32 tile, 128 p, 16 chunk, 128]
    xt = x.reshape(NT, DIM).T  # [DIM, NT]
    xtt = np.ascontiguousarray(
        xt.reshape(16, 128, 32, 128).transpose(2, 1, 0, 3)
    ).astype(bf)

    cos = freqs_cis[:, :, 0]  # [SEQ, 64]
    sin = freqs_cis[:, :, 1]
    cosb = np.concatenate([cos] * B, axis=0)  # [NT, 64], row n = b*SEQ + pos
    sinb = np.concatenate([sin] * B, axis=0)
    cos2n = np.concatenate([cosb, cosb], axis=1)  # [NT, 128] dup halves
    sin2n = np.concatenate([sinb, sinb], axis=1)
    # partition-major for contiguous DMA: [128 p, 32 i, 128 f] flattened
    cos2 = cos2n.reshape(32, 128, 128).transpose(1, 0, 2).reshape(128, 32 * 128)
    sin2 = sin2n.reshape(32, 128, 128).transpose(1, 0, 2).reshape(128, 32 * 128)
    cos2 = np.ascontiguousarray(cos2).astype(bf)
    sin2 = np.ascontiguousarray(sin2).astype(bf)

    perm = np.concatenate([np.arange(0, DH, 2), np.arange(1, DH, 2)])  # de-interleave
    ident = np.eye(128, dtype=np.float32).astype(bf)
    in_maps = []
    def ptile(a, inner):  # [16*128, m] -> [128 p, 16 chunk, m]
        m = a.shape[1]
        return np.ascontiguousarray(
            a.reshape(inner, 128, m).transpose(1, 0, 2)
        ).astype(bf)

    for c in range(NCORES):
        h0 = c * HL
        wq = [wqkv[:, h * DH : (h + 1) * DH][:, perm] * SCALE for h in (h0, h0 + 1)]
        wk = [wqkv[:, DIM + h * DH : DIM + (h + 1) * DH][:, perm] for h in (h0, h0 + 1)]
        wqk_c = ptile(np.concatenate(wq + wk, axis=1), 16)  # [128, 16, 512]
        wv_c = ptile(wqkv[:, 2 * DIM + h0 * DH : 2 * DIM + (h0 + HL) * DH], 16)
        wo_c = ptile(wo[h0 * DH : (h0 + HL) * DH, :], HL)  # [128, 2, DIM]
        in_maps.append(
            {
                "xtt": xtt,
                "wqkt": wqk_c,
                "wvt": wv_c,
                "wot": wo_c,
                "cos2": cos2,
                "sin2": sin2,
                "ident": ident,
            }
        )
    return in_maps


def _run(in_maps, trace=False, **kw):
    from concourse.bass_utils import run_bass_kernel_spmd

    prog, _ = _get_prog()
    return run_bass_kernel_spmd(prog, in_maps, list(range(NCORES)), trace=trace, **kw)


def kernel(x, freqs_cis, wqkv, wo):
    _get_prog()
    in_maps = _shard(x, freqs_cis, wqkv, wo)
    res = _run(in_maps, trace=False)
    acc = np.zeros((NT, DIM), dtype=np.float32)
    for c in range(NCORES):
        acc += np.asarray(res.results[c]["out_p"], dtype=np.float32)
    return acc.reshape(B, SEQ, DIM)
